# revision 19
# baseline (speedup 1.0000x reference)
"""Multi-head self-attention 2d kernel for 8 trn2 NeuronCores.

Sharding: data-parallel over batch B=16 -> 2 batches per core.

All matmul operands are bf16 (fp32 PSUM accumulate). bf16 weights enable
FWL (fast weight load) and decoupled LDWEIGHTS that the PE pulls ahead of
in-flight matmuls; f32r fuses the weight load into each matmul and
serializes the PE at ~half rate. V tiles are padded to 128 weight columns
so AV weight loads behave like the (fast) 128x128 projection loads.

Per-core dataflow (per batch):
  xf [C=512 part, N=1024 free] bf16
  q = wq@xf + bq  -> [C, N] bf16     (psum f32, vector bias-add)
  k = wk@xf + bk  -> [C, N] bf16
  vT = xf.T@wvT + bv -> v tiles [N, 8, 128] bf16 (64 d + ones row + zero pad)
  per head pair hp (software-pipelined one pair behind):
    eT[j, i] = k_h.T @ q_h           (bf16, K=64, row-tiled pairs overlap)
    expT = exp(SCALE * eT) bf16      (ACT; no max subtraction, |SCALE*e| < 8)
    out_u[0:65, i] = v_h.T @ expT    (bf16; accumulate over j; row 64=denom)
    r = 1/denom broadcast over 64 partitions via K=1 bf16 matmul
    out_norm = out_u[0:64] * r -> bf16
  y = gamma*(wo@out_norm + bo) + x   (bf16 matmul, bf16 residual/output)

The scalar engine (exp: 128 tiles x ~1.05us) is the per-pair pacing
engine; QK/AV/projection/out-projection tensor work and vector work are
scheduled to fit inside each pair's exp budget via a filler queue.
"""

import sys

for _p in ("/opt/trn_rl_repo",):
    if _p not in sys.path:
        sys.path.insert(0, _p)

import numpy as np
import ml_dtypes

import concourse.bass as bass
from concourse import bacc
import concourse.mybir as mybir
import concourse.tile as tile
from concourse.bass_utils import run_bass_kernel_spmd

F32 = mybir.dt.float32
F32R = mybir.dt.float32r
BF16 = mybir.dt.bfloat16
F8 = mybir.dt.float8e5
U8 = mybir.dt.uint8
AF = mybir.ActivationFunctionType
ALU = mybir.AluOpType
DR = mybir.MatmulPerfMode.DoubleRow

C = 512
N = 1024
HEADS = 8
HD = C // HEADS  # 64
SCALE = HD ** -0.5
CT = C // 128  # 4 channel tiles
NT = N // 128  # 8 spatial tiles
NTP = NT // 2  # 4 spatial tile pairs (fp8 DoubleRow k-tile pairs)
NCH = N // 512  # 2 free-dim chunks
BPC = 2  # batches per core
NCORES = 8
VW = HD + 2  # v tile row stride per head: 64 d + ones + pad (66B, 16B-aligned
             # ko step 8*66=528)
C_BIAS = 2.0  # exp bias so fp8e5m2 covers the weight range; cancels in norm
A_Q = (HD ** -0.5) * np.log2(np.e) * 4.0  # exp2-bits scale folded into q


def build_program():
    nc = bacc.Bacc(trn_type="TRN2", target_bir_lowering=False, debug=False,
                   num_devices=NCORES)

    x2 = nc.dram_tensor("x2", [BPC, C, N], BF16, kind="ExternalInput").ap()
    wT = {
        name: nc.dram_tensor(name, [C, C], BF16, kind="ExternalInput").ap()
        for name in ("wqT", "wkT", "wvT", "woT")
    }
    bq_r = nc.dram_tensor("bq_r", [128, CT], F32, kind="ExternalInput").ap()
    bk_r = nc.dram_tensor("bk_r", [128, CT], F32, kind="ExternalInput").ap()
    bo_r = nc.dram_tensor("bo_r", [128, CT], F32, kind="ExternalInput").ap()
    bv = nc.dram_tensor("bv", [C], F32, kind="ExternalInput").ap()
    gamma = nc.dram_tensor("gamma", [1], F32, kind="ExternalInput").ap()
    ones64h = nc.dram_tensor("ones64h", [HD], BF16, kind="ExternalInput").ap()
    y2 = nc.dram_tensor("y2", [BPC, C, N], BF16, kind="ExternalOutput").ap()

    with tile.TileContext(nc) as tc:
        with (
            tc.tile_pool(name="sb", bufs=1) as sb,
            tc.tile_pool(name="ps", bufs=1, space="PSUM") as ps,
        ):
            st = {"xf": {}, "vext": {}, "on": {},
                  "q": {0: [None] * CT, 1: [None] * CT},
                  "k": {0: [None] * CT, 1: [None] * CT}}

            def load_x(b):
                tiles = []
                for ct in range(CT):
                    t = sb.tile([128, N], BF16, tag=f"xf{ct}", bufs=2,
                                name=f"xf{b}_{ct}")
                    nc.sync.dma_start(out=t,
                                      in_=x2[b, ct * 128:(ct + 1) * 128, :])
                    tiles.append(t)
                st["xf"][b] = tiles

            load_x(0)

            w_sb = {}
            _dmae = [nc.scalar, nc.gpsimd, nc.sync]
            _di = 0
            for name in ("wqT", "wkT", "wvT", "woT"):
                tiles = []
                for kc in range(CT):
                    t = sb.tile([128, C], BF16, tag=f"{name}{kc}")
                    _dmae[_di % 3].dma_start(
                        out=t, in_=wT[name][kc * 128:(kc + 1) * 128, :])
                    _di += 1
                    tiles.append(t)
                w_sb[name] = tiles

            bq_sb = sb.tile([128, CT], F32, tag="bq")
            nc.gpsimd.dma_start(out=bq_sb, in_=bq_r)
            bk_sb = sb.tile([128, CT], F32, tag="bk")
            nc.gpsimd.dma_start(out=bk_sb, in_=bk_r)
            bo_sb = sb.tile([128, CT], F32, tag="bo")
            nc.gpsimd.dma_start(out=bo_sb, in_=bo_r)
            bv_bc = sb.tile([128, C], F32, tag="bv")
            nc.gpsimd.dma_start(
                out=bv_bc,
                in_=bass.AP(tensor=bv.tensor, offset=bv.offset,
                            ap=[[0, 128]] + list(bv.ap)))
            gam_sb = sb.tile([128, 1], F32, tag="gam")
            nc.gpsimd.dma_start(
                out=gam_sb,
                in_=bass.AP(tensor=gamma.tensor, offset=gamma.offset,
                            ap=[[0, 128]] + list(gamma.ap)))
            nbias_sb = sb.tile([128, 1], F32, tag="nbias")
            nc.gpsimd.memset(nbias_sb, -C_BIAS)
            ones1 = sb.tile([1, HD], BF16, tag="ones1")
            nc.gpsimd.dma_start(
                out=ones1,
                in_=bass.AP(tensor=ones64h.tensor, offset=ones64h.offset,
                            ap=[[0, 1]] + list(ones64h.ap)))

            # v tiles: [128 j, 2 ko, 8 h, VW] fp8e4m3 per jt-pair; per head:
            # 64 d values, the ones row (denominator trick) at 64, pad at 65.
            for bb in range(BPC):
                for ntp in range(NTP):
                    t = sb.tile([128, 2, HEADS, VW], F8, tag=f"v{ntp}",
                                name=f"vext{bb}_{ntp}", bufs=2)
                    nc.gpsimd.memset(t[:, :, :, HD:HD + 1], 1.0)
                    nc.gpsimd.memset(t[:, :, :, HD + 1:VW], 0.0)
                    st["vext"][(bb, ntp)] = t

            def proj_qk_chunk(b, wname, ot, nch):
                bias_sb, dstkey = (bq_sb, "q") if wname == "wqT" else (bk_sb, "k")
                if nch == 0:
                    st[dstkey][b][ot] = sb.tile(
                        [128, N], BF16, tag=f"{wname}o{ot}", bufs=2,
                        name=f"{dstkey}{b}_{ot}")
                t = st[dstkey][b][ot]
                p = ps.tile([128, 512], F32, tag="pq", bufs=2,
                            name=f"pj{b}{wname}{ot}{nch}")
                for kc in range(CT):
                    nc.tensor.matmul(
                        p,
                        lhsT=w_sb[wname][kc][:, ot * 128:(ot + 1) * 128],
                        rhs=st["xf"][b][kc][:, nch * 512:(nch + 1) * 512],
                        start=(kc == 0), stop=(kc == CT - 1),
                    )
                if dstkey == "q":
                    nc.vector.tensor_scalar(
                        t[:, nch * 512:(nch + 1) * 512], p,
                        bias_sb[:, ot:ot + 1], A_Q, ALU.add, ALU.mult)
                else:
                    nc.vector.tensor_scalar_add(
                        t[:, nch * 512:(nch + 1) * 512], p,
                        bias_sb[:, ot:ot + 1])

            def proj_qk_group(b, wname, ot):
                for nch in range(NCH):
                    proj_qk_chunk(b, wname, ot, nch)

            def proj_v_group(b, nt):
                p = ps.tile([128, 512], F32, tag="pq", bufs=2,
                            name=f"pv{b}{nt}")
                for kc in range(CT):
                    nc.tensor.matmul(
                        p,
                        lhsT=st["xf"][b][kc][:, nt * 128:(nt + 1) * 128],
                        rhs=w_sb["wvT"][kc],
                        start=(kc == 0), stop=(kc == CT - 1),
                    )
                nc.vector.tensor_tensor(
                    st["vext"][(b, nt // 2)][:, nt % 2, :, 0:HD],
                    p.rearrange("p (h d) -> p h d", h=HEADS),
                    bv_bc.rearrange("p (h d) -> p h d", h=HEADS),
                    ALU.add,
                )

            def alloc_on(b):
                st["on"][b] = [sb.tile([128, N], BF16, tag=f"on{ct}",
                                       name=f"on{b}_{ct}", bufs=2)
                               for ct in range(CT)]

            def outproj_group(b, ot, nch):
                p = ps.tile([128, 512], F32, tag="pq", bufs=2,
                            name=f"po{b}{ot}{nch}")
                for ctt in range(CT):
                    nc.tensor.matmul(
                        p,
                        lhsT=w_sb["woT"][ctt][:, ot * 128:(ot + 1) * 128],
                        rhs=st["on"][b][ctt][:, nch * 512:(nch + 1) * 512],
                        start=(ctt == 0), stop=(ctt == CT - 1),
                    )
                yt = sb.tile([128, 512], BF16, tag="y", bufs=2,
                             name=f"y{b}{ot}{nch}")
                nc.vector.tensor_scalar(
                    yt, p, bo_sb[:, ot:ot + 1], gam_sb[:, 0:1],
                    ALU.add, ALU.mult)
                nc.gpsimd.tensor_tensor(
                    yt, yt,
                    st["xf"][b][ot][:, nch * 512:(nch + 1) * 512],
                    ALU.add)
                nc.gpsimd.dma_start(
                    out=y2[b, ot * 128:(ot + 1) * 128,
                           nch * 512:(nch + 1) * 512],
                    in_=yt)

            # ---------- attention building blocks ----------
            EXP = {}  # (b, hp) -> [hh][jtp] fp8 expT tiles [128, 2, N]
            PU = {}   # (b, hp, hh) -> [pu_ic0, pu_ic1]

            # exp tiles computed on the vector engine (uint8/fp8e5m2 exp2
            # bit trick) instead of scalar ACT, to balance the two engines.
            # q is pre-scaled by A_Q so the bit pattern is psum + B_EXP2,
            # bottom-clamped via max; ACT path rescales by 1/A_Q.
            DVE_EXP = {(1, 0), (5, 1)}
            B_EXP2 = (15.0 - C_BIAS * np.log2(np.e) - 0.043) * 4.0
            ACT_SCALE = SCALE / A_Q

            def qk_exp(b, hp, jt):
                """4 QK matmuls (row-tiled head pair) + 2 exps for one jt."""
                q_sb, k_sb = st["q"][b], st["k"][b]
                jtp, ko = divmod(jt, 2)
                pe_pair = [ps.tile([128, N], F32, tag="pe", bufs=2,
                                   name=f"pe{b}_{hp}_{jt}_{hh}")
                           for hh in range(2)]
                for hh in range(2):
                    for ic in range(NCH):
                        nc.tensor.matmul(
                            pe_pair[hh][:, ic * 512:(ic + 1) * 512],
                            lhsT=k_sb[hp][hh * 64:(hh + 1) * 64,
                                          jt * 128:(jt + 1) * 128],
                            rhs=q_sb[hp][hh * 64:(hh + 1) * 64,
                                         ic * 512:(ic + 1) * 512],
                            start=True, stop=True,
                        )
                for hh in range(2):
                    if ko == 0:
                        e = sb.tile([128, 2, N], F8, tag="exp", bufs=16,
                                    name=f"e{b}_{hp}_{jtp}_{hh}")
                        EXP[(b, hp)][hh].append(e)
                    e = EXP[(b, hp)][hh][jtp]
                    if (jt, hh) in DVE_EXP:
                        nc.vector.tensor_scalar(
                            e.bitcast(U8)[:, ko, :], pe_pair[hh],
                            -B_EXP2, B_EXP2, ALU.max, ALU.add)
                    else:
                        nc.scalar.activation(e[:, ko, :], pe_pair[hh],
                                             AF.Exp, scale=ACT_SCALE,
                                             bias=nbias_sb[:, 0:1])

            def av_step(b, hp, hh, jtp):
                """One jt-pair DoubleRow step of the AV chain for one head."""
                h = 2 * hp + hh
                if jtp == 0:
                    PU[(b, hp, hh)] = [
                        ps.tile([128, 512], F32, tag="pu", bufs=2,
                                name=f"pu{b}_{h}_{ic}")
                        for ic in range(NCH)]
                pus = PU[(b, hp, hh)]
                expT = EXP[(b, hp)][hh]
                for ic in range(NCH):
                    nc.tensor.matmul(
                        pus[ic][0:HD + 1, :],
                        lhsT=st["vext"][(b, jtp)][:, :, h, 0:HD + 1],
                        rhs=expT[jtp][:, :, ic * 512:(ic + 1) * 512],
                        start=(jtp == 0), stop=(jtp == NTP - 1),
                        perf_mode=DR,
                        skip_group_check=True,
                    )

            def norm_tail(b, hp, hh):
                h = 2 * hp + hh
                on_sb = st["on"][b]
                ct, half = divmod(h, 2)
                for ic in range(NCH):
                    pu = PU[(b, hp, hh)][ic]
                    den = sb.tile([1, 512], BF16, tag="den", bufs=2,
                                  name=f"den{b}_{h}_{ic}")
                    nc.vector.tensor_copy(den, pu[HD:HD + 1, :])
                    rb = ps.tile([HD, 512], F32, tag="pq", bufs=2,
                                 name=f"rb{b}_{h}_{ic}")
                    nc.tensor.matmul(rb, lhsT=ones1, rhs=den,
                                     start=True, stop=True)
                    r_sb = sb.tile([HD, 512], F32, tag="rsb", bufs=2,
                                   name=f"r{b}_{h}_{ic}")
                    nc.vector.reciprocal_approx_fast(out=r_sb, in_=rb)
                    nc.vector.tensor_tensor(
                        on_sb[ct][half * 64:(half + 1) * 64,
                                  ic * 512:(ic + 1) * 512],
                        pu[0:HD, :], r_sb, ALU.mult)
                del PU[(b, hp, hh)]

            # ================= emission schedule =================
            # Filler queue: cheap groups scheduled into exp-paced slack.
            fillers = []

            def run_filler(n):
                for _ in range(n):
                    if fillers:
                        fillers.pop(0)()

            alloc_on(0)
            alloc_on(1)

            # head: q/k for heads 0,1 of batch 0 only, then attention starts
            proj_qk_group(0, "wqT", 0)
            proj_qk_group(0, "wkT", 0)

            # filler order obeys dependencies:
            #  pair (0,0): remaining b0 projections (q/k ot1 first - needed by
            #              pair (0,1) - then all b0 v tiles)
            fillers += [lambda ot=ot, w=w, nch=nch: proj_qk_chunk(0, w, ot, nch)
                        for ot in (1,) for w in ("wqT", "wkT")
                        for nch in range(NCH)]
            fillers += [lambda: load_x(1)]
            fillers += [lambda nt=nt: proj_v_group(0, nt) for nt in range(NT)]
            fillers += [lambda ot=ot, w=w, nch=nch: proj_qk_chunk(0, w, ot, nch)
                        for ot in (2, 3) for w in ("wqT", "wkT")
                        for nch in range(NCH)]
            #  pairs (0,1)-(0,3): b1 projections
            fillers += [lambda ot=ot, w=w, nch=nch: proj_qk_chunk(1, w, ot, nch)
                        for ot in range(CT) for w in ("wqT", "wkT")
                        for nch in range(NCH)]
            fillers += [lambda nt=nt: proj_v_group(1, nt) for nt in range(NT)]
            #  pairs (1,1)+: b0 out-projection (ready once AV(0,3) done)
            b0_op = [lambda ot=ot, nch=nch: outproj_group(0, ot, nch)
                     for ot in range(CT) for nch in range(NCH)]

            pairs = [(0, hp) for hp in range(4)] + [(1, hp) for hp in range(4)]
            prev = None
            for pi, (b, hp) in enumerate(pairs):
                EXP[(b, hp)] = [[], []]
                if pi == 5:
                    fillers += b0_op
                # per jt slot: non-blocking tensor work first (AV DoubleRow
                # step of the previous pair, filler group), then this pair's
                # QK whose pe-buffer wait gates the in-order tensor queue
                for jt in range(NT):
                    if prev is not None and jt % 2 == 1:
                        av_step(prev[0], prev[1], 0, jt // 2)
                    run_filler(1)
                    qk_exp(b, hp, jt)
                if prev is not None:
                    norm_tail(prev[0], prev[1], 0)
                    for jtp in range(NTP):
                        av_step(prev[0], prev[1], 1, jtp)
                        if jtp in (1, 3):
                            run_filler(1)
                    norm_tail(prev[0], prev[1], 1)
                    del EXP[prev]
                prev = (b, hp)

            for hh in range(2):
                for jtp in range(NTP):
                    av_step(1, 3, hh, jtp)
                norm_tail(1, 3, hh)
            run_filler(len(fillers))
            for ot in range(CT):
                for nch in range(NCH):
                    outproj_group(1, ot, nch)
    nc.compile()
    return nc


_PROGRAM = None


def _get_program():
    global _PROGRAM
    if _PROGRAM is None:
        _PROGRAM = build_program()
    return _PROGRAM


def kernel(**inputs):
    x = np.ascontiguousarray(inputs["x"], dtype=np.float32)
    B, c, H, W = x.shape
    assert (c, H * W) == (C, N)
    xr = x.reshape(B, C, N).astype(ml_dtypes.bfloat16)

    wqT = np.ascontiguousarray(inputs["wq"].T).astype(ml_dtypes.bfloat16)
    wkT = np.ascontiguousarray(inputs["wk"].T).astype(ml_dtypes.bfloat16)
    wvT = np.ascontiguousarray(inputs["wv"].T).astype(ml_dtypes.bfloat16)
    woT = np.ascontiguousarray(inputs["wo"].T).astype(ml_dtypes.bfloat16)
    bq_r = np.ascontiguousarray(inputs["bq"].astype(np.float32).reshape(CT, 128).T)
    bk_r = np.ascontiguousarray(inputs["bk"].astype(np.float32).reshape(CT, 128).T)
    bo_r = np.ascontiguousarray(inputs["bo"].astype(np.float32).reshape(CT, 128).T)
    bv = np.ascontiguousarray(inputs["bv"].astype(np.float32))
    gamma = np.ascontiguousarray(inputs["gamma"].astype(np.float32))

    shared = dict(wqT=wqT, wkT=wkT, wvT=wvT, woT=woT,
                  bq_r=bq_r, bk_r=bk_r, bo_r=bo_r, bv=bv, gamma=gamma,
                  ones64h=np.ones(HD, dtype=ml_dtypes.bfloat16))
    in_maps = []
    for core in range(NCORES):
        m = dict(shared)
        m["x2"] = np.ascontiguousarray(xr[core * BPC:(core + 1) * BPC])
        in_maps.append(m)

    nc = _get_program()
    res = run_bass_kernel_spmd(nc, in_maps, list(range(NCORES)))
    y = np.concatenate([res.results[i]["y2"].astype(np.float32)
                        for i in range(NCORES)], axis=0)
    return y.reshape(B, C, H, W)


if __name__ == "__main__":
    rng = np.random.default_rng(0)
    ins = {
        "x": rng.standard_normal((16, C, 32, 32), dtype=np.float32),
        "wq": rng.standard_normal((C, C), dtype=np.float32) / 23,
        "bq": rng.standard_normal((C,), dtype=np.float32) / 23,
        "wk": rng.standard_normal((C, C), dtype=np.float32) / 23,
        "bk": rng.standard_normal((C,), dtype=np.float32) / 23,
        "wv": rng.standard_normal((C, C), dtype=np.float32) / 23,
        "bv": rng.standard_normal((C,), dtype=np.float32) / 23,
        "wo": rng.standard_normal((C, C), dtype=np.float32) / 23,
        "bo": rng.standard_normal((C,), dtype=np.float32) / 23,
        "gamma": np.full((1,), 0.1, dtype=np.float32),
    }
    out = kernel(**ins)
    print("kernel ran, out shape", out.shape)


# revision 20
# speedup vs baseline: 1.1796x; 1.1796x over previous
"""Multi-head self-attention 2d kernel for 8 trn2 NeuronCores.

Sharding: data-parallel over batch B=16 -> 2 batches per core.

All matmul operands are bf16 (fp32 PSUM accumulate). bf16 weights enable
FWL (fast weight load) and decoupled LDWEIGHTS that the PE pulls ahead of
in-flight matmuls; f32r fuses the weight load into each matmul and
serializes the PE at ~half rate. V tiles are padded to 128 weight columns
so AV weight loads behave like the (fast) 128x128 projection loads.

Per-core dataflow (per batch):
  xf [C=512 part, N=1024 free] bf16
  q = wq@xf + bq  -> [C, N] bf16     (psum f32, vector bias-add)
  k = wk@xf + bk  -> [C, N] bf16
  vT = xf.T@wvT + bv -> v tiles [N, 8, 128] bf16 (64 d + ones row + zero pad)
  per head pair hp (software-pipelined one pair behind):
    eT[j, i] = k_h.T @ q_h           (bf16, K=64, row-tiled pairs overlap)
    expT = exp(SCALE * eT) bf16      (ACT; no max subtraction, |SCALE*e| < 8)
    out_u[0:65, i] = v_h.T @ expT    (bf16; accumulate over j; row 64=denom)
    r = 1/denom broadcast over 64 partitions via K=1 bf16 matmul
    out_norm = out_u[0:64] * r -> bf16
  y = gamma*(wo@out_norm + bo) + x   (bf16 matmul, bf16 residual/output)

The scalar engine (exp: 128 tiles x ~1.05us) is the per-pair pacing
engine; QK/AV/projection/out-projection tensor work and vector work are
scheduled to fit inside each pair's exp budget via a filler queue.
"""

import sys

for _p in ("/opt/trn_rl_repo",):
    if _p not in sys.path:
        sys.path.insert(0, _p)

import numpy as np
import ml_dtypes

import concourse.bass as bass
from concourse import bacc
import concourse.mybir as mybir
import concourse.tile as tile
from concourse.bass_utils import run_bass_kernel_spmd

F32 = mybir.dt.float32
F32R = mybir.dt.float32r
BF16 = mybir.dt.bfloat16
F8 = mybir.dt.float8e5
U8 = mybir.dt.uint8
AF = mybir.ActivationFunctionType
ALU = mybir.AluOpType
DR = mybir.MatmulPerfMode.DoubleRow

C = 512
N = 1024
HEADS = 8
HD = C // HEADS  # 64
SCALE = HD ** -0.5
CT = C // 128  # 4 channel tiles
NT = N // 128  # 8 spatial tiles
NTP = NT // 2  # 4 spatial tile pairs (fp8 DoubleRow k-tile pairs)
NCH = N // 512  # 2 free-dim chunks
BPC = 2  # batches per core
NCORES = 8
VW = HD + 2  # v tile row stride per head: 64 d + ones + pad (66B, 16B-aligned
             # ko step 8*66=528)
C_BIAS = 2.0  # exp bias so fp8e5m2 covers the weight range; cancels in norm
A_Q = (HD ** -0.5) * np.log2(np.e) * 4.0  # exp2-bits scale folded into q


def build_program():
    nc = bacc.Bacc(trn_type="TRN2", target_bir_lowering=False, debug=False,
                   num_devices=NCORES)

    x2 = nc.dram_tensor("x2", [BPC, C, N], BF16, kind="ExternalInput").ap()
    wT = {
        name: nc.dram_tensor(name, [C, C], BF16, kind="ExternalInput").ap()
        for name in ("wqT", "wkT", "wvT", "woT")
    }
    bq_r = nc.dram_tensor("bq_r", [128, CT], F32, kind="ExternalInput").ap()
    bk_r = nc.dram_tensor("bk_r", [128, CT], F32, kind="ExternalInput").ap()
    bo_r = nc.dram_tensor("bo_r", [128, CT], F32, kind="ExternalInput").ap()
    bv = nc.dram_tensor("bv", [C], F32, kind="ExternalInput").ap()
    gamma = nc.dram_tensor("gamma", [1], F32, kind="ExternalInput").ap()
    ones64h = nc.dram_tensor("ones64h", [HD], BF16, kind="ExternalInput").ap()
    y2 = nc.dram_tensor("y2", [BPC, C, N], BF16, kind="ExternalOutput").ap()

    with tile.TileContext(nc) as tc:
        with (
            tc.tile_pool(name="sb", bufs=1) as sb,
            tc.tile_pool(name="ps", bufs=1, space="PSUM") as ps,
        ):
            st = {"xf": {}, "vext": {}, "on": {},
                  "q": {0: [None] * CT, 1: [None] * CT},
                  "k": {0: [None] * CT, 1: [None] * CT}}

            def load_x(b):
                tiles = []
                for ct in range(CT):
                    t = sb.tile([128, N], BF16, tag=f"xf{ct}", bufs=2,
                                name=f"xf{b}_{ct}")
                    nc.sync.dma_start(out=t,
                                      in_=x2[b, ct * 128:(ct + 1) * 128, :])
                    tiles.append(t)
                st["xf"][b] = tiles

            load_x(0)

            w_sb = {}
            _dmae = [nc.scalar, nc.gpsimd, nc.sync]
            _di = 0
            for name in ("wqT", "wkT", "wvT", "woT"):
                tiles = []
                for kc in range(CT):
                    t = sb.tile([128, C], BF16, tag=f"{name}{kc}")
                    _dmae[_di % 3].dma_start(
                        out=t, in_=wT[name][kc * 128:(kc + 1) * 128, :])
                    _di += 1
                    tiles.append(t)
                w_sb[name] = tiles

            bq_sb = sb.tile([128, CT], F32, tag="bq")
            nc.gpsimd.dma_start(out=bq_sb, in_=bq_r)
            bk_sb = sb.tile([128, CT], F32, tag="bk")
            nc.gpsimd.dma_start(out=bk_sb, in_=bk_r)
            bo_sb = sb.tile([128, CT], F32, tag="bo")
            nc.gpsimd.dma_start(out=bo_sb, in_=bo_r)
            bv_bc = sb.tile([128, C], F32, tag="bv")
            nc.gpsimd.dma_start(
                out=bv_bc,
                in_=bass.AP(tensor=bv.tensor, offset=bv.offset,
                            ap=[[0, 128]] + list(bv.ap)))
            gam_sb = sb.tile([128, 1], F32, tag="gam")
            nc.gpsimd.dma_start(
                out=gam_sb,
                in_=bass.AP(tensor=gamma.tensor, offset=gamma.offset,
                            ap=[[0, 128]] + list(gamma.ap)))
            nbias_sb = sb.tile([128, 1], F32, tag="nbias")
            nc.gpsimd.memset(nbias_sb, -C_BIAS)
            ones1 = sb.tile([1, HD], BF16, tag="ones1")
            nc.gpsimd.dma_start(
                out=ones1,
                in_=bass.AP(tensor=ones64h.tensor, offset=ones64h.offset,
                            ap=[[0, 1]] + list(ones64h.ap)))

            # v tiles: [128 j, 2 ko, 8 h, VW] fp8e4m3 per jt-pair; per head:
            # 64 d values, the ones row (denominator trick) at 64, pad at 65.
            for bb in range(BPC):
                for ntp in range(NTP):
                    t = sb.tile([128, 2, HEADS, VW], F8, tag=f"v{ntp}",
                                name=f"vext{bb}_{ntp}", bufs=2)
                    nc.gpsimd.memset(t[:, :, :, HD:HD + 1], 1.0)
                    nc.gpsimd.memset(t[:, :, :, HD + 1:VW], 0.0)
                    st["vext"][(bb, ntp)] = t

            def proj_qk_chunk(b, wname, ot, nch):
                bias_sb, dstkey = (bq_sb, "q") if wname == "wqT" else (bk_sb, "k")
                if nch == 0:
                    st[dstkey][b][ot] = sb.tile(
                        [128, N], BF16, tag=f"{wname}o{ot}", bufs=2,
                        name=f"{dstkey}{b}_{ot}")
                t = st[dstkey][b][ot]
                p = ps.tile([128, 512], F32, tag="pq", bufs=2,
                            name=f"pj{b}{wname}{ot}{nch}")
                for kc in range(CT):
                    nc.tensor.matmul(
                        p,
                        lhsT=w_sb[wname][kc][:, ot * 128:(ot + 1) * 128],
                        rhs=st["xf"][b][kc][:, nch * 512:(nch + 1) * 512],
                        start=(kc == 0), stop=(kc == CT - 1),
                    )
                if dstkey == "q":
                    nc.vector.tensor_scalar(
                        t[:, nch * 512:(nch + 1) * 512], p,
                        bias_sb[:, ot:ot + 1], A_Q, ALU.add, ALU.mult)
                else:
                    nc.vector.tensor_scalar_add(
                        t[:, nch * 512:(nch + 1) * 512], p,
                        bias_sb[:, ot:ot + 1])

            def proj_qk_group(b, wname, ot):
                for nch in range(NCH):
                    proj_qk_chunk(b, wname, ot, nch)

            def proj_v_group(b, nt):
                p = ps.tile([128, 512], F32, tag="pq", bufs=2,
                            name=f"pv{b}{nt}")
                for kc in range(CT):
                    nc.tensor.matmul(
                        p,
                        lhsT=st["xf"][b][kc][:, nt * 128:(nt + 1) * 128],
                        rhs=w_sb["wvT"][kc],
                        start=(kc == 0), stop=(kc == CT - 1),
                    )
                nc.vector.tensor_tensor(
                    st["vext"][(b, nt // 2)][:, nt % 2, :, 0:HD],
                    p.rearrange("p (h d) -> p h d", h=HEADS),
                    bv_bc.rearrange("p (h d) -> p h d", h=HEADS),
                    ALU.add,
                )

            def alloc_on(b):
                st["on"][b] = [sb.tile([128, N], BF16, tag=f"on{ct}",
                                       name=f"on{b}_{ct}", bufs=2)
                               for ct in range(CT)]

            def outproj_group(b, ot, nch):
                p = ps.tile([128, 512], F32, tag="pq", bufs=2,
                            name=f"po{b}{ot}{nch}")
                for ctt in range(CT):
                    nc.tensor.matmul(
                        p,
                        lhsT=w_sb["woT"][ctt][:, ot * 128:(ot + 1) * 128],
                        rhs=st["on"][b][ctt][:, nch * 512:(nch + 1) * 512],
                        start=(ctt == 0), stop=(ctt == CT - 1),
                    )
                yt = sb.tile([128, 512], BF16, tag="y", bufs=2,
                             name=f"y{b}{ot}{nch}")
                nc.vector.tensor_scalar(
                    yt, p, bo_sb[:, ot:ot + 1], gam_sb[:, 0:1],
                    ALU.add, ALU.mult)
                nc.vector.tensor_tensor(
                    yt, yt,
                    st["xf"][b][ot][:, nch * 512:(nch + 1) * 512],
                    ALU.add)
                nc.gpsimd.dma_start(
                    out=y2[b, ot * 128:(ot + 1) * 128,
                           nch * 512:(nch + 1) * 512],
                    in_=yt)

            # ---------- attention building blocks ----------
            EXP = {}  # (b, hp) -> [hh][jtp] fp8 expT tiles [128, 2, N]
            PU = {}   # (b, hp, hh) -> [pu_ic0, pu_ic1]

            # exp tiles computed on the vector engine (uint8/fp8e5m2 exp2
            # bit trick) instead of scalar ACT, to balance the two engines.
            # q is pre-scaled by A_Q so the bit pattern is psum + B_EXP2,
            # bottom-clamped via max; ACT path rescales by 1/A_Q.
            DVE_EXP = {(1, 0), (5, 1)}
            B_EXP2 = (15.0 - C_BIAS * np.log2(np.e) - 0.043) * 4.0
            ACT_SCALE = SCALE / A_Q

            def qk_exp(b, hp, jt):
                """4 QK matmuls (row-tiled head pair) + 2 exps for one jt."""
                q_sb, k_sb = st["q"][b], st["k"][b]
                jtp, ko = divmod(jt, 2)
                pe_pair = [ps.tile([128, N], F32, tag="pe", bufs=2,
                                   name=f"pe{b}_{hp}_{jt}_{hh}")
                           for hh in range(2)]
                for ic in range(NCH):
                    for hh in range(2):
                        nc.tensor.matmul(
                            pe_pair[hh][:, ic * 512:(ic + 1) * 512],
                            lhsT=k_sb[hp][hh * 64:(hh + 1) * 64,
                                          jt * 128:(jt + 1) * 128],
                            rhs=q_sb[hp][hh * 64:(hh + 1) * 64,
                                         ic * 512:(ic + 1) * 512],
                            start=True, stop=True,
                        )
                for hh in range(2):
                    if ko == 0:
                        e = sb.tile([128, 2, N], F8, tag="exp", bufs=16,
                                    name=f"e{b}_{hp}_{jtp}_{hh}")
                        EXP[(b, hp)][hh].append(e)
                    e = EXP[(b, hp)][hh][jtp]
                    if (jt, hh) in DVE_EXP:
                        nc.vector.tensor_scalar(
                            e.bitcast(U8)[:, ko, :], pe_pair[hh],
                            -B_EXP2, B_EXP2, ALU.max, ALU.add)
                    else:
                        nc.scalar.activation(e[:, ko, :], pe_pair[hh],
                                             AF.Exp, scale=ACT_SCALE,
                                             bias=nbias_sb[:, 0:1])

            def av_step(b, hp, hh, jtp):
                """One jt-pair DoubleRow step of the AV chain for one head."""
                h = 2 * hp + hh
                if jtp == 0:
                    PU[(b, hp, hh)] = [
                        ps.tile([128, 512], F32, tag="pu", bufs=2,
                                name=f"pu{b}_{h}_{ic}")
                        for ic in range(NCH)]
                pus = PU[(b, hp, hh)]
                expT = EXP[(b, hp)][hh]
                for ic in range(NCH):
                    nc.tensor.matmul(
                        pus[ic][0:HD + 1, :],
                        lhsT=st["vext"][(b, jtp)][:, :, h, 0:HD + 1],
                        rhs=expT[jtp][:, :, ic * 512:(ic + 1) * 512],
                        start=(jtp == 0), stop=(jtp == NTP - 1),
                        perf_mode=DR,
                        skip_group_check=True,
                    )

            def norm_tail(b, hp, hh):
                h = 2 * hp + hh
                on_sb = st["on"][b]
                ct, half = divmod(h, 2)
                for ic in range(NCH):
                    pu = PU[(b, hp, hh)][ic]
                    den = sb.tile([1, 512], BF16, tag="den", bufs=2,
                                  name=f"den{b}_{h}_{ic}")
                    nc.vector.tensor_copy(den, pu[HD:HD + 1, :])
                    rb = ps.tile([HD, 512], F32, tag="pq", bufs=2,
                                 name=f"rb{b}_{h}_{ic}")
                    nc.tensor.matmul(rb, lhsT=ones1, rhs=den,
                                     start=True, stop=True)
                    r_sb = sb.tile([HD, 512], F32, tag="rsb", bufs=2,
                                   name=f"r{b}_{h}_{ic}")
                    nc.vector.reciprocal_approx_fast(out=r_sb, in_=rb)
                    nc.vector.tensor_tensor(
                        on_sb[ct][half * 64:(half + 1) * 64,
                                  ic * 512:(ic + 1) * 512],
                        pu[0:HD, :], r_sb, ALU.mult)
                del PU[(b, hp, hh)]

            # ================= emission schedule =================
            # Filler queue: cheap groups scheduled into exp-paced slack.
            fillers = []

            def run_filler(n):
                for _ in range(n):
                    if fillers:
                        fillers.pop(0)()

            alloc_on(0)
            alloc_on(1)

            # head: q/k for heads 0,1 of batch 0 only, then attention starts
            proj_qk_group(0, "wqT", 0)
            proj_qk_group(0, "wkT", 0)

            # filler order obeys dependencies:
            #  pair (0,0): remaining b0 projections (q/k ot1 first - needed by
            #              pair (0,1) - then all b0 v tiles)
            fillers += [lambda ot=ot, w=w, nch=nch: proj_qk_chunk(0, w, ot, nch)
                        for ot in (1,) for w in ("wqT", "wkT")
                        for nch in range(NCH)]
            fillers += [lambda: load_x(1)]
            fillers += [lambda nt=nt: proj_v_group(0, nt) for nt in range(NT)]
            fillers += [lambda ot=ot, w=w, nch=nch: proj_qk_chunk(0, w, ot, nch)
                        for ot in (2, 3) for w in ("wqT", "wkT")
                        for nch in range(NCH)]
            #  pairs (0,1)-(0,3): b1 projections
            fillers += [lambda ot=ot, w=w, nch=nch: proj_qk_chunk(1, w, ot, nch)
                        for ot in range(CT) for w in ("wqT", "wkT")
                        for nch in range(NCH)]
            fillers += [lambda nt=nt: proj_v_group(1, nt) for nt in range(NT)]
            #  pairs (1,1)+: b0 out-projection (ready once AV(0,3) done)
            b0_op = [lambda ot=ot, nch=nch: outproj_group(0, ot, nch)
                     for ot in range(CT) for nch in range(NCH)]

            pairs = [(0, hp) for hp in range(4)] + [(1, hp) for hp in range(4)]
            prev = None
            for pi, (b, hp) in enumerate(pairs):
                EXP[(b, hp)] = [[], []]
                if pi == 5:
                    fillers += b0_op
                # per jt slot: non-blocking tensor work first (AV DoubleRow
                # step of the previous pair, filler group), then this pair's
                # QK whose pe-buffer wait gates the in-order tensor queue
                for jt in range(NT):
                    if prev is not None and jt % 2 == 1:
                        av_step(prev[0], prev[1], 0, jt // 2)
                    run_filler(1)
                    qk_exp(b, hp, jt)
                if prev is not None:
                    norm_tail(prev[0], prev[1], 0)
                    for jtp in range(NTP):
                        av_step(prev[0], prev[1], 1, jtp)
                        if jtp in (1, 3):
                            run_filler(1)
                    norm_tail(prev[0], prev[1], 1)
                    del EXP[prev]
                prev = (b, hp)

            for hh in range(2):
                for jtp in range(NTP):
                    av_step(1, 3, hh, jtp)
                norm_tail(1, 3, hh)
            run_filler(len(fillers))
            for ot in range(CT):
                for nch in range(NCH):
                    outproj_group(1, ot, nch)
    nc.compile()
    return nc


_PROGRAM = None


def _get_program():
    global _PROGRAM
    if _PROGRAM is None:
        _PROGRAM = build_program()
    return _PROGRAM


def kernel(**inputs):
    x = np.ascontiguousarray(inputs["x"], dtype=np.float32)
    B, c, H, W = x.shape
    assert (c, H * W) == (C, N)
    xr = x.reshape(B, C, N).astype(ml_dtypes.bfloat16)

    wqT = np.ascontiguousarray(inputs["wq"].T).astype(ml_dtypes.bfloat16)
    wkT = np.ascontiguousarray(inputs["wk"].T).astype(ml_dtypes.bfloat16)
    wvT = np.ascontiguousarray(inputs["wv"].T).astype(ml_dtypes.bfloat16)
    woT = np.ascontiguousarray(inputs["wo"].T).astype(ml_dtypes.bfloat16)
    bq_r = np.ascontiguousarray(inputs["bq"].astype(np.float32).reshape(CT, 128).T)
    bk_r = np.ascontiguousarray(inputs["bk"].astype(np.float32).reshape(CT, 128).T)
    bo_r = np.ascontiguousarray(inputs["bo"].astype(np.float32).reshape(CT, 128).T)
    bv = np.ascontiguousarray(inputs["bv"].astype(np.float32))
    gamma = np.ascontiguousarray(inputs["gamma"].astype(np.float32))

    shared = dict(wqT=wqT, wkT=wkT, wvT=wvT, woT=woT,
                  bq_r=bq_r, bk_r=bk_r, bo_r=bo_r, bv=bv, gamma=gamma,
                  ones64h=np.ones(HD, dtype=ml_dtypes.bfloat16))
    in_maps = []
    for core in range(NCORES):
        m = dict(shared)
        m["x2"] = np.ascontiguousarray(xr[core * BPC:(core + 1) * BPC])
        in_maps.append(m)

    nc = _get_program()
    res = run_bass_kernel_spmd(nc, in_maps, list(range(NCORES)))
    y = np.concatenate([res.results[i]["y2"].astype(np.float32)
                        for i in range(NCORES)], axis=0)
    return y.reshape(B, C, H, W)


if __name__ == "__main__":
    rng = np.random.default_rng(0)
    ins = {
        "x": rng.standard_normal((16, C, 32, 32), dtype=np.float32),
        "wq": rng.standard_normal((C, C), dtype=np.float32) / 23,
        "bq": rng.standard_normal((C,), dtype=np.float32) / 23,
        "wk": rng.standard_normal((C, C), dtype=np.float32) / 23,
        "bk": rng.standard_normal((C,), dtype=np.float32) / 23,
        "wv": rng.standard_normal((C, C), dtype=np.float32) / 23,
        "bv": rng.standard_normal((C,), dtype=np.float32) / 23,
        "wo": rng.standard_normal((C, C), dtype=np.float32) / 23,
        "bo": rng.standard_normal((C,), dtype=np.float32) / 23,
        "gamma": np.full((1,), 0.1, dtype=np.float32),
    }
    out = kernel(**ins)
    print("kernel ran, out shape", out.shape)
